# revision 1
# baseline (speedup 1.0000x reference)
"""PILCO GP world-model moment matching on 8 TRN2 NeuronCores.

Strategy
--------
The O(B*D^2*N^2) part of the reference (the per-pair Q = exp(...) matrices of
size N x N = 512 x 512, contracted with beta / inv_K) runs on device; all the
small E x E (10x10) linear algebra runs on host in float64.

Algebraic reduction: R(a,d) = Sigma * (iLam_a + iLam_d) + I is symmetric in
(a, d) and S = R^-1 Sigma is a symmetric matrix, so Q_(d,a) = Q_(a,d)^T and
only the 36 pairs a <= d are needed (instead of 64 ordered pairs).

Per (pair, batch) job on device (N = 512, E = 10):
  arg_ij = ca_i + (Pa S)_i . Pd_j + cb'_j   -- one K=12 augmented matmul
           A_aug = [U, ca, 1] (stationary), B_aug = [Pd, 1, cb'] (moving)
  E = exp(arg)                              -- ScalarE, 1024-wide PSUM reads
  s_j = sum_i beta_a_i E_ij                 -- M=1 accumulating matmuls
  diag jobs also: r_i = sum_j E_ij invK_ij  -- DVE mult + reduce
Host finishes rows = s . beta_d and the trace sums, then assembles
(mu_next, Sigma_next).

576 jobs are sharded 72 per core; each core's 72 jobs are packed 3 per
128-partition block (base partitions 0/32/64) for full-bandwidth DMA.
Matmuls run in float32r (full-rate fp32 streaming on the PE).
"""

import os
import numpy as np

B, D, F_, N = 16, 8, 2, 512
E = D + F_
K_AUG = 12
NJOBS = 72          # per core
NBLK = 24           # 3 jobs per 128-partition block
NCORES = 8

# diag job positions within the per-core job list (same for every core; the
# device program hardcodes which jobs carry the inv_K reduction)
DIAG_POS = [int(i * NJOBS / 16) for i in range(16)]
_diag_set = set(DIAG_POS)
assert len(DIAG_POS) == 16

_COMPILED = None
LAST_RESULTS = None  # stashed BassKernelResults for test harness introspection


def _build_program():
    import concourse.bacc as bacc
    import concourse.tile as tile
    from concourse import mybir

    F32 = mybir.dt.float32
    F32R = mybir.dt.float32r
    AF = mybir.ActivationFunctionType
    ALU = mybir.AluOpType

    nc = bacc.Bacc("TRN2", target_bir_lowering=False, debug=False)
    jA = nc.dram_tensor("jA", [128, NBLK * 512], F32R, kind="ExternalInput")
    jB = nc.dram_tensor("jB", [128, NBLK * 512], F32R, kind="ExternalInput")
    jbeta = nc.dram_tensor("jbeta", [128, NJOBS * 4], F32R, kind="ExternalInput")
    invk = nc.dram_tensor("invk", [128, 2048], F32, kind="ExternalInput")
    s_out = nc.dram_tensor("s_out", [NJOBS, 512], F32, kind="ExternalOutput")
    r_out = nc.dram_tensor("r_out", [128, 64], F32, kind="ExternalOutput")

    with tile.TileContext(nc) as tc:
        with tc.tile_pool(name="inp", bufs=1) as inp, \
             tc.tile_pool(name="psm", bufs=3, space="PSUM") as psm, \
             tc.tile_pool(name="pss", bufs=2, space="PSUM") as pss, \
             tc.tile_pool(name="epool", bufs=3) as epool, \
             tc.tile_pool(name="wrk", bufs=3) as wrk:
            jA_sb = inp.tile([128, NBLK * 512], F32R)
            jB_sb = inp.tile([128, NBLK * 512], F32R)
            jbeta_sb = inp.tile([128, NJOBS * 4], F32R)
            invk_sb = inp.tile([128, 2048], F32)
            r_sb = inp.tile([128, 64], F32)

            # chunked input DMAs so early jobs can start before all data lands
            for blk in range(NBLK):
                sl = slice(blk * 512, (blk + 1) * 512)
                nc.sync.dma_start(jA_sb[:, sl], jA[:, sl])
                nc.sync.dma_start(jB_sb[:, sl], jB[:, sl])
            nc.sync.dma_start(jbeta_sb[:], jbeta[:])
            nc.sync.dma_start(invk_sb[:], invk[:])

            # software-pipelined by one job: emit job k's matmuls+exp, then
            # job k-1's contraction stage, so PE always has m-matmuls ready
            # while waiting on ACT.
            pending = None  # (k, e_sb, diag_idx or None)

            def emit_contraction(k, e_sb, diag_idx):
                ps_s = pss.tile([1, 512], mybir.dt.float32, tag="s")
                for t in range(4):
                    nc.tensor.matmul(
                        ps_s[:],
                        jbeta_sb[:, k * 4 + t:k * 4 + t + 1],
                        e_sb[:, t * 512:(t + 1) * 512],
                        start=(t == 0), stop=(t == 3),
                        skip_group_check=True,
                    )
                s_stage = wrk.tile([1, 512], mybir.dt.float32, tag="ss")
                nc.vector.tensor_copy(s_stage[:], ps_s[:])
                nc.sync.dma_start(s_out[k:k + 1, :], s_stage[:])
                if diag_idx is not None:
                    for t in range(4):
                        scratch = wrk.tile([128, 512], mybir.dt.float32, tag="scr")
                        nc.vector.tensor_tensor(
                            scratch[:],
                            e_sb[:, t * 512:(t + 1) * 512].bitcast(mybir.dt.float32),
                            invk_sb[:, t * 512:(t + 1) * 512],
                            ALU.mult,
                        )
                        nc.vector.tensor_reduce(
                            r_sb[:, diag_idx * 4 + t:diag_idx * 4 + t + 1],
                            scratch[:], mybir.AxisListType.X, ALU.add,
                        )

            diag_counter = 0
            for k in range(NJOBS):
                blk, g = k // 3, k % 3
                rows = slice(32 * g, 32 * g + K_AUG)
                bcols = slice(blk * 512, (blk + 1) * 512)
                e_sb = epool.tile([128, 2048], F32R, tag="e")
                for h in range(2):
                    ps_m = psm.tile([128, 1024], mybir.dt.float32, tag="m")
                    for tt in range(2):
                        t = 2 * h + tt
                        nc.tensor.matmul(
                            ps_m[:, tt * 512:(tt + 1) * 512],
                            jA_sb[rows, blk * 512 + t * 128: blk * 512 + (t + 1) * 128],
                            jB_sb[rows, bcols],
                            start=True, stop=True,
                            skip_group_check=True,
                        )
                    nc.scalar.activation(
                        e_sb[:, h * 1024:(h + 1) * 1024], ps_m[:], AF.Exp,
                    )
                if pending is not None:
                    emit_contraction(*pending)
                if k in _diag_set:
                    pending = (k, e_sb, diag_counter)
                    diag_counter += 1
                else:
                    pending = (k, e_sb, None)
            emit_contraction(*pending)
            nc.sync.dma_start(r_out[:], r_sb[:])

    nc.compile()
    return nc


def _job_lists():
    """Per-core job list: list of (a, d, b) in device job order."""
    pairs_off = [(a, d) for a in range(D) for d in range(a + 1, D)]  # 28
    jobs_off_all = [(a, d, b) for (a, d) in pairs_off for b in range(B)]  # 448
    per_core = []
    for c in range(NCORES):
        off = jobs_off_all[c * 56:(c + 1) * 56]
        jobs = [None] * NJOBS
        for i, p in enumerate(DIAG_POS):
            jobs[p] = (c, c, i)  # diag job i -> batch i
        it = iter(off)
        for k in range(NJOBS):
            if jobs[k] is None:
                jobs[k] = next(it)
        per_core.append(jobs)
    return per_core


def kernel(obs_mean, obs_var, action_mean, action_var, cross_cov,
           X_train, ell, alpha_sq, sigma_sq_eps, beta, inv_K):
    global _COMPILED, LAST_RESULTS
    from concourse.bass_utils import run_bass_kernel_spmd

    f8 = np.float64
    obs_mean64 = obs_mean.astype(f8)
    Xt = X_train.astype(f8)
    ell64 = ell.astype(f8)
    alpha64 = alpha_sq.astype(f8)
    beta64 = beta.astype(f8)

    mu = np.concatenate([obs_mean64, action_mean.astype(f8)], axis=-1)      # [B,E]
    top = np.concatenate([obs_var.astype(f8), cross_cov.astype(f8)], axis=-1)
    bot = np.concatenate([np.swapaxes(cross_cov.astype(f8), -1, -2),
                          action_var.astype(f8)], axis=-1)
    Sigma = np.concatenate([top, bot], axis=-2)                             # [B,E,E]

    nu = Xt[None, :, :] - mu[:, None, :]                                    # [B,N,E]
    ell2 = ell64 ** 2
    iLam = 1.0 / ell2                                                       # [D,E]
    Lam = np.zeros((D, E, E))
    Lam[:, np.arange(E), np.arange(E)] = ell2

    # ---- predictive mean + input-output covariance (host)
    Amat = Sigma[:, None, :, :] + Lam[None, :, :, :]                        # [B,D,E,E]
    Ainv = np.linalg.inv(Amat)
    logdetA = np.linalg.slogdet(Amat)[1]                                    # [B,D]
    logdetLam = np.sum(np.log(ell2), axis=-1)                               # [D]
    quad = np.einsum('bne,bdef,bnf->bdn', nu, Ainv, nu, optimize=True)
    q = alpha64[None, :, None] * np.exp(
        0.5 * (logdetLam[None, :] - logdetA)[..., None] - 0.5 * quad)       # [B,D,N]
    mu_delta = np.einsum('dn,bdn->bd', beta64, q)
    w = np.einsum('dn,bdn,bne->bde', beta64, q, nu, optimize=True)
    V_io = np.einsum('bef,bdfg,bdg->bde', Sigma, Ainv, w, optimize=True)    # [B,D,E]

    logk = np.log(alpha64)[None, :, None] - 0.5 * np.einsum(
        'bne,de->bdn', nu ** 2, iLam, optimize=True)                        # [B,D,N]
    P = nu[:, None, :, :] * iLam[None, :, None, :]                          # [B,D,N,E]

    # ---- per-pair quantities for a <= d (vectorized over pairs x batches)
    pairs = [(a, d) for a in range(D) for d in range(a, D)]                 # 36
    a_idx = np.array([p[0] for p in pairs])
    d_idx = np.array([p[1] for p in pairs])
    iLsum = iLam[a_idx] + iLam[d_idx]                                       # [36,E]
    R = Sigma[None, :, :, :] * iLsum[:, None, None, :] + np.eye(E)          # [36,B,E,E]
    S = np.linalg.solve(R, np.broadcast_to(Sigma[None], R.shape))
    logdetR = np.linalg.slogdet(R)[1]                                       # [36,B]
    Pa = np.ascontiguousarray(P[:, a_idx].transpose(1, 0, 2, 3))            # [36,B,N,E]
    Pd = np.ascontiguousarray(P[:, d_idx].transpose(1, 0, 2, 3))
    U = np.einsum('pbne,pbef->pbnf', Pa, S, optimize=True)
    Vd = np.einsum('pbne,pbef->pbnf', Pd, S, optimize=True)
    d_a = (U * Pa).sum(-1)                                                  # [36,B,N]
    d_b = (Vd * Pd).sum(-1)
    ca = logk[:, a_idx].transpose(1, 0, 2) + 0.5 * d_a                      # [36,B,N]
    cb = (logk[:, d_idx].transpose(1, 0, 2) + 0.5 * d_b
          - 0.5 * logdetR[..., None])                                       # [36,B,N]

    pair_no = {p: i for i, p in enumerate(pairs)}

    A_aug = np.empty((36, B, K_AUG, N), np.float32)   # [12, N] = A_aug^T
    A_aug[:, :, :E, :] = U.transpose(0, 1, 3, 2)
    A_aug[:, :, E, :] = ca
    A_aug[:, :, E + 1, :] = 1.0
    B_aug = np.empty((36, B, K_AUG, N), np.float32)
    B_aug[:, :, :E, :] = Pd.transpose(0, 1, 3, 2)
    B_aug[:, :, E, :] = 1.0
    B_aug[:, :, E + 1, :] = cb

    per_core_jobs = _job_lists()
    beta32 = beta.astype(np.float32)
    in_maps = []
    for c in range(NCORES):
        jobs = per_core_jobs[c]
        jA_h = np.zeros((128, NBLK * 512), np.float32)
        jB_h = np.zeros((128, NBLK * 512), np.float32)
        jbeta_h = np.zeros((128, NJOBS * 4), np.float32)
        for k, (a, d, b) in enumerate(jobs):
            blk, g = k // 3, k % 3
            p = pair_no[(a, d)]
            jA_h[32 * g:32 * g + K_AUG, blk * 512:(blk + 1) * 512] = A_aug[p, b]
            jB_h[32 * g:32 * g + K_AUG, blk * 512:(blk + 1) * 512] = B_aug[p, b]
            jbeta_h[:, k * 4:(k + 1) * 4] = beta32[a].reshape(4, 128).T
        invk_h = np.concatenate(
            [inv_K[c, t * 128:(t + 1) * 128, :] for t in range(4)],
            axis=1).astype(np.float32)
        in_maps.append({"jA": jA_h, "jB": jB_h, "jbeta": jbeta_h, "invk": invk_h})

    if _COMPILED is None:
        _COMPILED = _build_program()
    nc = _COMPILED

    trace = bool(int(os.environ.get("GP_KERNEL_TRACE", "0")))
    res = run_bass_kernel_spmd(nc, in_maps, core_ids=list(range(NCORES)),
                               trace=trace)
    LAST_RESULTS = res

    # ---- host postprocessing
    EDD = np.zeros((B, D, D))
    tr = np.zeros((B, D))
    for c in range(NCORES):
        s_all = res.results[c]["s_out"].astype(f8)      # [72,512]
        r_all = res.results[c]["r_out"].astype(f8)      # [128,64]
        diag_counter = 0
        for k, (a, d, b) in enumerate(per_core_jobs[c]):
            val = s_all[k] @ beta64[d]
            EDD[b, a, d] = val
            EDD[b, d, a] = val
            if k in _diag_set:
                tr[b, a] = r_all[:, diag_counter * 4:(diag_counter + 1) * 4].sum()
                diag_counter += 1

    Sigma_delta = EDD - mu_delta[:, :, None] * mu_delta[:, None, :]
    diag_corr = alpha64[None, :] - tr + sigma_sq_eps.astype(f8)[None, :]
    Sigma_delta = Sigma_delta + np.einsum(
        'bd,de->bde', diag_corr, np.eye(D), optimize=True)

    C_xd = np.swapaxes(V_io[..., :D], 1, 2)                                 # [B,D,D]
    mu_next = obs_mean64 + mu_delta
    Sigma_next = obs_var.astype(f8) + Sigma_delta + C_xd + np.swapaxes(C_xd, -1, -2)
    return mu_next.astype(np.float32), Sigma_next.astype(np.float32)
